# revision 6
# baseline (speedup 1.0000x reference)
"""Trainium2 Bass kernel for nn_Attention_25262997635932.

Reference computes:
    energy[s,b,g] = sum_h enc[s,b,h] * W[g,h] + bias[g]
    scores[s,b]   = sum_g energy[s,b,g] * hidden[b,g]
    attn          = softmax(scores, axis=batch)        -> [1, 1, S, B]

Reassociated (exact in real arithmetic):
    v[b,h] = sum_g hidden[b,g] * W[g,h]       (= hidden @ W, tiny)
    c[b]   = sum_g bias[g] * hidden[b,g]
    scores[s,b] = sum_h enc[s,b,h] * v[b,h] + c[b]

Distribution over 8 cores: shard the h (hidden_size) axis. Core i gets
W[:, i*512:(i+1)*512] (8 MiB) and enc[:, :, i*512:(i+1)*512] (16 MiB),
computes v for its h-columns with TensorE (W is the moving operand),
reduces enc·v per position with fused DVE multiply-reduce, then one
32 KiB AllReduce of the per-position partial scores; softmax over the
batch pair runs redundantly on every core.
"""

import numpy as np

S, B, H, NCORES = 4096, 2, 4096, 8
HS = H // NCORES  # 512 h-columns per core
ST = 128  # s positions per partition tile
NT = S // ST  # 32 s-tiles
TPC = 4  # s-tiles per enc DMA chunk (2 MiB)
NCHUNK = NT // TPC
GO = H // 128  # 32 contraction chunks for v
WCH = 4  # go-chunks per W DMA chunk (1 MiB)
NWCH = GO // WCH

_CACHE = {}


def _split_overfull_waits(nc, max_waits=1):
    """This neuronx-cc build's CTRL codegen accepts a single sync-wait per
    instruction, but the Tile kernel-tail drain carries one wait per used
    proc lane. Hoist extras onto preceding same-engine NoOps."""
    import concourse.mybir as mybir

    for bb in nc.main_func.blocks:
        new_list = []
        for ins in bb.instructions:
            si = ins.sync_info
            ow = list(si.on_wait) if (si is not None and si.on_wait) else []
            if len(ow) > max_waits:
                extras = ow[:-max_waits]
                keep = ow[-max_waits:]
                for k, w in enumerate(extras):
                    nop = mybir.InstNoOp(
                        name=f"{ins.name}-waitsplit-{k}", ins=[], outs=[]
                    )
                    nop.engine = ins.engine
                    nop.sync_info = mybir.SyncInfo(on_wait=[w], on_update=[])
                    new_list.append(nop)
                ins.sync_info = mybir.SyncInfo(
                    on_wait=keep, on_update=list(si.on_update)
                )
            new_list.append(ins)
        bb.instructions[:] = new_list


def _build():
    import concourse.bass as bass
    import concourse.mybir as mybir
    from concourse import tile

    f32 = mybir.dt.float32
    alu = mybir.AluOpType
    nc = bass.Bass(num_devices=NCORES)

    enc_ext = nc.declare_dram_parameter("enc", [S, B, HS], f32, isOutput=False)
    w_ext = nc.declare_dram_parameter("w", [H, HS], f32, isOutput=False)
    hid_ext = nc.declare_dram_parameter("hidt", [128, GO, B], f32, isOutput=False)
    cb_ext = nc.declare_dram_parameter("cb", [128, B], f32, isOutput=False)
    out_ext = nc.declare_dram_parameter("out", [128, NT * B], f32, isOutput=True)

    with tile.TileContext(nc) as tc:
        with (
            tc.tile_pool(name="small", bufs=1) as small,
            tc.tile_pool(name="wpool", bufs=3) as wpool,
            tc.tile_pool(name="epool", bufs=3) as epool,
            tc.tile_pool(name="scratch", bufs=2) as scratch,
            tc.tile_pool(name="psum", bufs=1, space="PSUM") as psum,
            tc.tile_pool(name="dram", bufs=1, space="DRAM") as dram,
        ):
            hid_t = small.tile([128, GO, B], f32)
            nc.sync.dma_start(hid_t[:], hid_ext[:])
            cb_t = small.tile([128, B], f32)
            nc.sync.dma_start(cb_t[:], cb_ext[:])

            # Phase 1: v[b, h'] = sum_g hidden[b, g] * W[g, h'].
            # hid chunk [gi=128, b] is the stationary, W chunk the moving
            # operand; accumulate the 32 g-chunks into one PSUM tile.
            v_ps = psum.tile([B, HS], f32)
            w_view = w_ext[:].rearrange("(nw wc gi) h -> nw gi wc h", wc=WCH, gi=128)
            for iw in range(NWCH):
                w_t = wpool.tile([128, WCH, HS], f32)
                nc.sync.dma_start(w_t[:], w_view[iw])
                for j in range(WCH):
                    go = iw * WCH + j
                    nc.tensor.matmul(
                        v_ps[:, :],
                        hid_t[:, go, :],
                        w_t[:, j, :],
                        start=(go == 0),
                        stop=(go == GO - 1),
                    )
            # Broadcast v across all 128 partitions (PSUM -> SBUF -> DRAM ->
            # stride-0 DMA back) so DVE can use it per-lane.
            v_sb = small.tile([B, HS], f32)
            nc.vector.tensor_copy(v_sb[:], v_ps[:])
            v_dram = dram.tile([B, HS], f32)
            nc.sync.dma_start(v_dram[:], v_sb[:])
            v_bc = small.tile([128, B, HS], f32)
            nc.sync.dma_start(v_bc[:], v_dram[:].partition_broadcast(128))

            # Phase 2: partial scores via fused multiply-reduce over h'.
            # sc_sb[p, t*B+b] = sum_h' enc[t*128+p, b, h'] v[b, h']
            sc_sb = small.tile([128, NT * B], f32)
            enc_view = enc_ext[:].rearrange(
                "(ch t p) b h -> ch p t b h", t=TPC, p=128
            )
            for ch in range(NCHUNK):
                e_t = epool.tile([128, TPC, B, HS], f32)
                nc.sync.dma_start(e_t[:], enc_view[ch])
                for j in range(TPC):
                    t_idx = ch * TPC + j
                    for b in range(B):
                        tt = scratch.tile([128, HS], f32, tag="ttr")
                        f = t_idx * B + b
                        nc.vector.scalar_tensor_tensor(
                            out=tt[:],
                            in0=e_t[:, j, b, :],
                            scalar=0.0,
                            in1=v_bc[:, b, :],
                            op0=alu.bypass,
                            op1=alu.mult,
                            accum_out=sc_sb[:, f : f + 1],
                        )

            # Phase 3: sum partial scores across the 8 cores.
            sc_in = dram.tile([128, NT * B], f32)
            sc_out = dram.tile([128, NT * B], f32, addr_space="Shared")
            nc.sync.dma_start(sc_in[:], sc_sb[:])
            nc.gpsimd.collective_compute(
                "AllReduce",
                alu.add,
                replica_groups=[list(range(NCORES))],
                ins=[sc_in.opt()],
                outs=[sc_out.opt()],
            )

            # Phase 4: softmax over the batch pair (axis of size 2).
            x = small.tile([128, NT * B], f32)
            nc.sync.dma_start(x[:], sc_out[:])
            xr = x[:].rearrange("p (t b) -> p t b", b=B)
            # Add the bias-path constant c[b] = bias . hidden[b] per batch.
            x0 = small.tile([128, NT], f32)
            x1 = small.tile([128, NT], f32)
            nc.vector.tensor_scalar_add(x0[:], xr[:, :, 0], cb_t[:, 0:1])
            nc.vector.tensor_scalar_add(x1[:], xr[:, :, 1], cb_t[:, 1:2])
            x0, x1 = x0[:], x1[:]
            mx = small.tile([128, NT], f32)
            nc.vector.tensor_tensor(mx[:], x0, x1, alu.max)
            d0 = small.tile([128, NT], f32)
            d1 = small.tile([128, NT], f32)
            nc.vector.tensor_sub(d0[:], x0, mx[:])
            nc.vector.tensor_sub(d1[:], x1, mx[:])
            e0 = small.tile([128, NT], f32)
            e1 = small.tile([128, NT], f32)
            nc.scalar.activation(e0[:], d0[:], mybir.ActivationFunctionType.Exp)
            nc.scalar.activation(e1[:], d1[:], mybir.ActivationFunctionType.Exp)
            ssum = small.tile([128, NT], f32)
            nc.vector.tensor_add(ssum[:], e0[:], e1[:])
            rec = small.tile([128, NT], f32)
            nc.vector.reciprocal(rec[:], ssum[:])
            o = small.tile([128, NT * B], f32)
            orr = o[:].rearrange("p (t b) -> p t b", b=B)
            nc.vector.tensor_mul(orr[:, :, 0], e0[:], rec[:])
            nc.vector.tensor_mul(orr[:, :, 1], e1[:], rec[:])
            nc.sync.dma_start(out_ext[:], o[:])

    _split_overfull_waits(nc)
    return nc


def kernel(hidden, encoder_outputs, W, b):
    from concourse.bass_utils import run_bass_kernel_spmd

    if "nc" not in _CACHE:
        _CACHE["nc"] = _build()
    nc = _CACHE["nc"]

    hidden = np.asarray(hidden, dtype=np.float32)
    enc = np.asarray(encoder_outputs, dtype=np.float32)
    W = np.asarray(W, dtype=np.float32)
    bias = np.asarray(b, dtype=np.float32)

    hid2 = hidden.reshape(B, H)
    # hidt[p, go, b] = hidden[b, go*128 + p]
    hidt = np.ascontiguousarray(hid2.T.reshape(GO, 128, B).transpose(1, 0, 2))
    c = hid2 @ bias  # [B], the bias-path constant
    cb = np.ascontiguousarray(np.tile(c[None, :], (128, 1)).astype(np.float32))

    in_maps = []
    for i in range(NCORES):
        sl = slice(i * HS, (i + 1) * HS)
        in_maps.append(
            {
                "enc": np.ascontiguousarray(enc[:, :, sl]),
                "w": np.ascontiguousarray(W[:, sl]),
                "hidt": hidt,
                "cb": cb,
            }
        )

    res = run_bass_kernel_spmd(nc, in_maps, core_ids=list(range(NCORES)))
    out = res.results[0]["out"]  # [128, NT*B], layout [p, t, b], s = t*128 + p
    attn = out.reshape(128, NT, B).transpose(1, 0, 2).reshape(S, B)
    return np.ascontiguousarray(attn[None, None]).astype(np.float32)
